# revision 75
# baseline (speedup 1.0000x reference)
"""AttentionFuserV3 Trainium2 kernel: 8-core pure data parallel over batch.

Reference computation per batch item x_b [L=1024, D=512]:
  stage1: q = x W1^T; S = q x^T; A = softmax(S); mix = A x;
          h = tanh([mix, q] Wo1^T); h = h / max(||h||_2, eps)     (per row)
  stage2: c = [h, x]; q2 = c W2^T; S2 = q2 c^T; A2 = softmax(S2);
          mix2 = A2 c; o = [mix2, q2] Wo2^T; emb = mean_l(o)

Pooling algebra: emb = mean_l(o) is linear, so the full [L,2D] mix2 and
[L,D] output projection are never materialized.  Instead
  emb = [colsum(A2) c, colsum(q2)] (Wo2^T / L)
where colsum(A2)[m] = sum_l exp(S2[l,m]) / denom[l] is a cheap
multiply+reduce over the already-computed exp tiles.  This removes the
two largest matmul groups of stage 2.

Layout strategy ("T-space"): all big tensors are kept transposed in SBUF
(feature dim on partitions, sequence dim L on the free axis) so every
matmul contraction lands on the partition axis.  Softmax runs without
max-subtraction (|scores| < ~70, exp stays in range); the denominator is
accumulated with a ones-vector matmul and applied as a column broadcast
produced by a rank-1 matmul.

All matmul operands are bf16 (same 1 cycle/row PE speed as f32r, half
the SBUF/DMA); accumulation stays in f32 PSUM.  The halved SBUF lets
every per-batch tile be double-buffered, and the program is emitted as
an explicit software pipeline: stage 2 of batch b is interleaved with
stage 1 of batch b+1 at phase granularity (and stage-1 phases alternate
their two l-chunks) so the in-order PE queue always has independent
matmuls between a producer phase and its consumer.
"""

import sys

sys.path.insert(0, "/opt/trn_rl_repo")

import numpy as np

N_GLOBAL, L, D = 32, 1024, 512
NCORES = 8
B = N_GLOBAL // NCORES          # 4 batch items per core
P = 128
LC = 512                        # l-chunk (matmul moving free dim)
NLC = L // LC                   # 2
DT = D // P                     # 4
LT = L // P                     # 8
D2T = 2 * D // P                # 8
C2T = 4 * D // P                # 16

_CACHE = {}


def _build_nc():
    import concourse.bass as bass  # noqa: F401
    import concourse.mybir as mybir
    import concourse.tile as tile
    from concourse import bacc

    f32 = mybir.dt.float32
    bf16 = mybir.dt.bfloat16
    AF = mybir.ActivationFunctionType
    ALU = mybir.AluOpType

    nc = bacc.Bacc("TRN2", target_bir_lowering=False, debug=False,
                   num_devices=NCORES)

    x_ext = nc.declare_dram_parameter("x", [B, L, D], bf16, isOutput=False)
    xT_ext = nc.declare_dram_parameter("xT", [B, D, L], bf16, isOutput=False)
    w1t_ext = nc.declare_dram_parameter("w1t", [D, D], bf16, isOutput=False)
    wo1t_ext = nc.declare_dram_parameter("wo1t", [2 * D, D], bf16, isOutput=False)
    w2t_ext = nc.declare_dram_parameter("w2t", [2 * D, 2 * D], bf16, isOutput=False)
    wo2t_ext = nc.declare_dram_parameter("wo2t", [4 * D, D], bf16, isOutput=False)
    id_ext = nc.declare_dram_parameter("ident", [P, P], bf16, isOutput=False)
    onc_ext = nc.declare_dram_parameter("onesc", [P, 1], bf16, isOutput=False)
    onr_ext = nc.declare_dram_parameter("onesr", [1, P], bf16, isOutput=False)
    out_ext = nc.declare_dram_parameter("out", [B, D], f32, isOutput=True)

    import time as _time
    _t0 = _time.time()
    with tile.TileContext(nc) as tc:
        with tc.tile_pool(name="wp", bufs=1) as wp, \
             tc.tile_pool(name="cp", bufs=1) as cp, \
             tc.tile_pool(name="xp", bufs=2) as xp, \
             tc.tile_pool(name="hp", bufs=2) as hp, \
             tc.tile_pool(name="tp", bufs=2) as tp, \
             tc.tile_pool(name="vp", bufs=2) as vp, \
             tc.tile_pool(name="ps", bufs=8, space="PSUM") as pp:

            # ---- per-batch tile state (input DMAs issued before the bulky
            # weight DMAs so ph1 of batch 0 can start early) ----
            T = [dict() for _ in range(B)]

            def start_batch(b, defer_x=False):
                t = T[b]
                t["xT"] = xp.tile([P, DT, L], bf16, tag="xT", name=f"xT_{b}")
                nc.sync.dma_start(out=t["xT"],
                                  in_=xT_ext[b].rearrange("(k p) l -> p k l", p=P))
                if not defer_x:
                    start_x(b)
                t["hTn"] = hp.tile([P, DT, L], bf16, tag="hTn", name=f"hTn_{b}")
                t["qT"] = [None] * NLC
                t["mixT"] = [None] * NLC
                t["expT"] = [None] * NLC
                t["hT"] = [None] * NLC
                t["q2T"] = [None] * NLC
                t["exp2"] = [None] * NLC
                t["rv1"] = [None] * NLC
                t["rv2"] = [None] * NLC
                t["rv3"] = [None] * NLC
                t["bc3x"] = [None] * NLC
                t["a2sr"] = [None] * NLC

            def start_x(b):
                t = T[b]
                t["x"] = xp.tile([P, LT, D], bf16, tag="x", name=f"x_{b}")
                nc.sync.dma_start(out=t["x"],
                                  in_=x_ext[b].rearrange("(k p) d -> p k d", p=P))

            start_batch(0, defer_x=True)

            # ---- weights + constants (resident) ----
            w1t_s = wp.tile([P, DT, D], bf16, tag="w1t")
            nc.sync.dma_start(out=w1t_s, in_=w1t_ext.rearrange("(k p) e -> p k e", p=P))
            ones_s = cp.tile([P, 1], bf16, tag="ones")
            nc.sync.dma_start(out=ones_s, in_=onc_ext[:, :])
            onesr_s = cp.tile([1, P], bf16, tag="onesr")
            nc.sync.dma_start(out=onesr_s, in_=onr_ext[:, :])
            start_x(0)
            wo1t_s = wp.tile([P, D2T, D], bf16, tag="wo1t")
            nc.sync.dma_start(out=wo1t_s, in_=wo1t_ext.rearrange("(k p) e -> p k e", p=P))
            ident_s = cp.tile([P, P], bf16, tag="ident")
            nc.sync.dma_start(out=ident_s, in_=id_ext[:, :])
            w2t_s = wp.tile([P, D2T, 2 * D], bf16, tag="w2t")
            nc.sync.dma_start(out=w2t_s, in_=w2t_ext.rearrange("(k p) e -> p k e", p=P))
            wo2t_s = wp.tile([P, C2T, D], bf16, tag="wo2t")
            nc.sync.dma_start(out=wo2t_s, in_=wo2t_ext.rearrange("(k p) e -> p k e", p=P))

            def mm(out, lhsT, rhs, first, last):
                nc.tensor.matmul(out, lhsT, rhs, start=first, stop=last)

            def recip_part(denom_ps, nm, clamp_eps=None):
                """[1,512] PSUM denominator -> [1,512] SBUF reciprocal
                (optionally sqrt+clamp first)."""
                rv = vp.tile([1, LC], bf16, tag="rv", bufs=3, name=f"rv_{nm}")
                with nc.allow_low_precision(reason="bf16 softmax scale"):
                    if clamp_eps is not None:
                        nv = vp.tile([1, LC], f32, tag="nv", bufs=1,
                                     name=f"nv_{nm}")
                        nc.scalar.sqrt(nv, denom_ps[0:1, :])
                        nc.vector.tensor_scalar_max(nv, nv, clamp_eps)
                        nc.vector.reciprocal(rv, nv)
                    else:
                        nc.vector.reciprocal(rv, denom_ps[0:1, :])
                return rv

            def bcast_part(rv, nm):
                """[1,512] reciprocal -> [128,512] broadcast via rank-1 matmul.
                Emitted a block after recip_part so the PE never waits on the
                DVE reciprocal."""
                ps_b = pp.tile([P, LC], f32, tag="ps", name=f"psb_{nm}")
                mm(ps_b, onesr_s, rv[0:1, :], True, True)
                bc = vp.tile([P, LC], f32, tag="bc", bufs=3, name=f"bc_{nm}")
                nc.scalar.copy(bc, ps_b)
                return bc

            # ================= stage 1 phases =================
            def ph1(b, lc):
                t = T[b]
                ls = slice(lc * LC, (lc + 1) * LC)
                qT = tp.tile([P, DT, LC], bf16, tag="qt", name=f"qT_{b}_{lc}")
                t["qT"][lc] = qT
                for et in range(DT):
                    ps = pp.tile([P, LC], f32, tag="ps", name=f"ps1_{b}_{lc}_{et}")
                    for dk in range(DT):
                        mm(ps, w1t_s[:, dk, et * P:(et + 1) * P],
                           t["xT"][:, dk, ls], dk == 0, dk == DT - 1)
                    nc.scalar.copy(qT[:, et, :], ps)

            def ph2(b, lc):
                t = T[b]
                expT = tp.tile([P, LT, LC], bf16, tag="exp", bufs=3,
                               name=f"expT_{b}_{lc}")
                t["expT"][lc] = expT
                ps_d = pp.tile([P, LC], f32, tag="ps", name=f"psd1_{b}_{lc}")
                for mt in range(LT):
                    ps = pp.tile([P, LC], f32, tag="ps", name=f"ps2_{b}_{lc}_{mt}")
                    for ek in range(DT):
                        mm(ps, t["xT"][:, ek, mt * P:(mt + 1) * P],
                           t["qT"][lc][:, ek, :], ek == 0, ek == DT - 1)
                    nc.scalar.activation(expT[:, mt, :], ps, AF.Exp)
                    mm(ps_d[0:1, :], ones_s, expT[:, mt, :],
                       mt == 0, mt == LT - 1)
                t["rv1"][lc] = recip_part(ps_d, f"b1_{b}_{lc}")

            def ph3(b, lc):
                t = T[b]
                bc1 = bcast_part(t["rv1"][lc], f"b1_{b}_{lc}")
                mixT = tp.tile([P, DT, LC], bf16, tag="mix", name=f"mixT_{b}_{lc}")
                t["mixT"][lc] = mixT
                ps_m = [pp.tile([P, LC], f32, tag="ps", name=f"psm_{b}_{lc}_{i}")
                        for i in range(DT)]
                for mk in range(LT):
                    for dt in range(DT):
                        mm(ps_m[dt], t["x"][:, mk, dt * P:(dt + 1) * P],
                           t["expT"][lc][:, mk, :], mk == 0, mk == LT - 1)
                for dt in range(DT):
                    nc.vector.tensor_mul(mixT[:, dt, :], ps_m[dt], bc1)

            def ph4(b, lc):
                t = T[b]
                hT = tp.tile([P, DT, LC], bf16, tag="ht", name=f"hT_{b}_{lc}")
                t["hT"][lc] = hT
                for ot in range(DT):
                    ps = pp.tile([P, LC], f32, tag="ps", name=f"ps4_{b}_{lc}_{ot}")
                    for ck in range(D2T):
                        rhs = (t["mixT"][lc][:, ck, :] if ck < DT
                               else t["qT"][lc][:, ck - DT, :])
                        mm(ps, wo1t_s[:, ck, ot * P:(ot + 1) * P],
                           rhs, ck == 0, ck == D2T - 1)
                    nc.scalar.activation(hT[:, ot, :], ps, AF.Tanh)

            def ph5a(b, lc):
                t = T[b]
                hsq = tp.tile([P, DT, LC], bf16, tag="hsq", name=f"hsq_{b}_{lc}")
                for dt in range(DT):
                    nc.scalar.activation(hsq[:, dt, :], t["hT"][lc][:, dt, :],
                                         AF.Square)
                ps_n = pp.tile([P, LC], f32, tag="ps", name=f"psn_{b}_{lc}")
                for dt in range(DT):
                    mm(ps_n[0:1, :], ones_s, hsq[:, dt, :], dt == 0, dt == DT - 1)
                t["rv2"][lc] = recip_part(ps_n, f"b2_{b}_{lc}", clamp_eps=1e-12)

            def ph5b(b, lc):
                t = T[b]
                ls = slice(lc * LC, (lc + 1) * LC)
                bc2 = bcast_part(t["rv2"][lc], f"b2_{b}_{lc}")
                for dt in range(DT):
                    nc.vector.tensor_mul(t["hTn"][:, dt, ls], t["hT"][lc][:, dt, :],
                                         bc2)

            # ================= stage 2 phases =================
            def c2T(t, k, fslice):
                """combined2T[d2,·] k-tile: [hTn; xT]"""
                return (t["hTn"][:, k, fslice] if k < DT
                        else t["xT"][:, k - DT, fslice])

            def ph7(b, lc):
                t = T[b]
                ls = slice(lc * LC, (lc + 1) * LC)
                if lc == 0:
                    t["q2r"] = vp.tile([P, D2T, NLC], f32, tag="q2r", bufs=2,
                                       name=f"q2r_{b}")
                q2T = tp.tile([P, D2T, LC], bf16, tag="q2", name=f"q2T_{b}_{lc}")
                t["q2T"][lc] = q2T
                for et in range(D2T):
                    ps = pp.tile([P, LC], f32, tag="ps", name=f"ps7_{b}_{lc}_{et}")
                    for dk in range(D2T):
                        mm(ps, w2t_s[:, dk, et * P:(et + 1) * P],
                           c2T(t, dk, ls), dk == 0, dk == D2T - 1)
                    nc.scalar.copy(q2T[:, et, :], ps)
                    nc.vector.tensor_reduce(t["q2r"][:, et, lc:lc + 1], ps,
                                            axis=mybir.AxisListType.X,
                                            op=ALU.add)

            def ph8a(b, lc):
                t = T[b]
                if lc == 0:
                    t["a2p"] = vp.tile([P, LT, NLC], f32, tag="a2p", bufs=2,
                                       name=f"a2p_{b}")
                    t["scr"] = vp.tile([P, LC], f32, tag="scr", bufs=1,
                                       name=f"scr_{b}")
                exp2 = tp.tile([P, LT, LC], bf16, tag="exp", bufs=3,
                                name=f"exp2_{b}_{lc}")
                t["exp2"][lc] = exp2
                ps_d = pp.tile([P, LC], f32, tag="ps", name=f"psd2_{b}_{lc}")
                for mt in range(LT):
                    ps = pp.tile([P, LC], f32, tag="ps", name=f"ps8_{b}_{lc}_{mt}")
                    for ek in range(D2T):
                        mm(ps, c2T(t, ek, slice(mt * P, (mt + 1) * P)),
                           t["q2T"][lc][:, ek, :], ek == 0, ek == D2T - 1)
                    nc.scalar.activation(exp2[:, mt, :], ps, AF.Exp)
                    mm(ps_d[0:1, :], ones_s, exp2[:, mt, :],
                       mt == 0, mt == LT - 1)
                t["rv3"][lc] = recip_part(ps_d, f"b3_{b}_{lc}")

            def ph8b(b, lc):
                # A2 column sums a2p[m,lc] = sum_{l in chunk} exp2[m,l]/denom[l]
                # via fused multiply+reduce on the DVE
                t = T[b]
                bc3 = bcast_part(t["rv3"][lc], f"b3_{b}_{lc}")
                t["bc3x"][lc] = bc3

            def ph8c(b, lc):
                t = T[b]
                bc3 = t["bc3x"][lc]
                for mt in range(LT):
                    nc.vector.affine_mul_reduce(
                        out=t["scr"], accum_out=t["a2p"][:, mt, lc:lc + 1],
                        in0=t["exp2"][lc][:, mt, :], in1=bc3,
                        scale=1.0, bias=0.0)
                if lc == 1:
                    # combine the chunk partials right behind the amr chain so
                    # epi_mid's transposes never wait on later DVE work
                    a2s = vp.tile([P, LT], f32, tag="a2s", bufs=2,
                                  name=f"a2s_{b}")
                    nc.vector.tensor_add(a2s, t["a2p"][:, :, 0],
                                         t["a2p"][:, :, 1])
                    t["a2sc"] = vp.tile([P, LT], bf16, tag="a2sc", bufs=2,
                                        name=f"a2sc_{b}")
                    nc.scalar.copy(t["a2sc"], a2s)

            def epi_mid(b):
                # comb = [colsum(A2) @ c, colsum(q2)] as a [4D] column vector.
                # colsum(A2) is transposed to a row (tiny PE transposes),
                # broadcast over partitions (rank-1 matmuls), then contracted
                # against the T-space tiles of c = [hn, x] with fused
                # multiply+reduce -- no natural-layout hidden, no DRAM bounce.
                t = T[b]
                a2sc = t["a2sc"]
                a2row = vp.tile([1, L], bf16, tag="a2row", bufs=2,
                                name=f"a2row_{b}")
                for mt in range(LT):
                    ps_t = pp.tile([1, P], bf16, tag="ps", name=f"pst_{b}_{mt}")
                    nc.tensor.transpose(ps_t, a2sc[:, mt:mt + 1], ident_s)
                    nc.scalar.copy(a2row[0:1, mt * P:(mt + 1) * P], ps_t)
                a2b = vp.tile([P, L], bf16, tag="a2b", bufs=2, name=f"a2b_{b}")
                for lc in range(NLC):
                    ls = slice(lc * LC, (lc + 1) * LC)
                    ps_bc = pp.tile([P, LC], f32, tag="ps",
                                    name=f"psbc_{b}_{lc}")
                    mm(ps_bc, onesr_s, a2row[0:1, ls], True, True)
                    nc.scalar.copy(a2b[:, ls], ps_bc)
                comb = vp.tile([P, C2T], f32, tag="comb", bufs=2, name=f"comb_{b}")
                t["comb"] = comb
                nc.vector.tensor_add(comb[:, D2T:C2T], t["q2r"][:, :, 0],
                                     t["q2r"][:, :, 1])
                scrL = vp.tile([P, L], bf16, tag="scrL", bufs=1,
                               name=f"scrL_{b}")
                for dt in range(DT):
                    nc.vector.affine_mul_reduce(
                        out=scrL, accum_out=comb[:, dt:dt + 1],
                        in0=t["hTn"][:, dt, :], in1=a2b, scale=1.0, bias=0.0)
                for dt in range(DT):
                    nc.vector.affine_mul_reduce(
                        out=scrL, accum_out=comb[:, DT + dt:DT + dt + 1],
                        in0=t["xT"][:, dt, :], in1=a2b, scale=1.0, bias=0.0)

            def epiB(b):
                # emb = comb @ (Wo2^T/L) as a single [1,D] row (1/L folded
                # into wo2t on the host)
                t = T[b]
                combr = vp.tile([P, C2T], bf16, tag="combr", bufs=2,
                                name=f"combr_{b}")
                nc.scalar.copy(combr, t["comb"])
                ps_o = pp.tile([1, D], f32, tag="ps", name=f"pso_{b}")
                for ck in range(C2T):
                    mm(ps_o[0:1, :], combr[:, ck:ck + 1], wo2t_s[:, ck, :],
                       ck == 0, ck == C2T - 1)
                orow = vp.tile([1, D], f32, tag="orow", bufs=2, name=f"orow_{b}")
                nc.scalar.copy(orow, ps_o)
                nc.sync.dma_start(out=out_ext[b:b + 1, :], in_=orow[0:1, :])

            # ================= emission schedule =================
            # Software pipeline: stage 2 of batch b (A-blocks) interleaved
            # with stage 1 of batch b+1 (B-blocks); epiB(b) is deferred into
            # iteration b+1 so the DRAM bounce is in flight under ph7.
            def S1(b):
                return [lambda lc=lc, f=f: f(b, lc)
                        for f in (ph1, ph2, ph3) for lc in range(NLC)
                        ] + [lambda: ph4(b, 0), lambda: ph5a(b, 0),
                             lambda: ph4(b, 1), lambda: ph5a(b, 1),
                             lambda: ph5b(b, 0), lambda: ph5b(b, 1)]

            def S2(b):
                return [lambda: ph7(b, 0),
                        lambda: ph8a(b, 0),
                        lambda: (ph7(b, 1), ph8b(b, 0), ph8c(b, 0)),
                        lambda: ph8a(b, 1),
                        lambda: ph8b(b, 1),
                        lambda: ph8c(b, 1),
                        lambda: epi_mid(b)]

            for f in S1(0):
                f()
            for b in range(B):
                A = S2(b)
                A[0]()
                if b > 0:
                    epiB(b - 1)
                if b + 1 < B:
                    start_batch(b + 1)
                    Bl = S1(b + 1)
                    Bl[0](); Bl[1]()
                    A[1]()
                    Bl[2](); Bl[3]()
                    A[2]()
                    Bl[4](); Bl[5]()
                    A[3]()
                    Bl[6]()
                    A[4]()          # bc3(b,1) broadcast matmul only
                    Bl[7]()         # ph5a(b+1,0): its reciprocal enqueues
                    A[5]()          # ...before the amr chain on the DVE
                    Bl[8](); Bl[9](); Bl[10](); Bl[11]()
                    A[6]()
                else:
                    for f in A[1:]:
                        f()
            epiB(B - 1)

    _t1 = _time.time()
    nc.compile()
    print(f"[kernel] tile-trace+schedule {_t1 - _t0:.1f}s, "
          f"bacc compile {_time.time() - _t1:.1f}s", file=sys.stderr, flush=True)
    return nc


def get_nc():
    if "nc" not in _CACHE:
        _CACHE["nc"] = _build_nc()
    return _CACHE["nc"]


def make_in_maps(x, W1, Wo1, W2, Wo2):
    import ml_dtypes
    bf = ml_dtypes.bfloat16
    x = np.ascontiguousarray(np.asarray(x, dtype=np.float32))
    xT = np.ascontiguousarray(x.transpose(0, 2, 1)).astype(bf)
    xb = x.astype(bf)
    w1t = np.ascontiguousarray(np.asarray(W1, np.float32).T).astype(bf)
    wo1t = np.ascontiguousarray(np.asarray(Wo1, np.float32).T).astype(bf)
    w2t = np.ascontiguousarray(np.asarray(W2, np.float32).T).astype(bf)
    # 1/L mean-pooling factor folded into the stage-2 output projection
    wo2t = (np.ascontiguousarray(np.asarray(Wo2, np.float32).T)
            * (1.0 / L)).astype(bf)
    ident = np.eye(P, dtype=np.float32).astype(bf)
    onesc = np.ones((P, 1), dtype=np.float32).astype(bf)
    onesr = np.ones((1, P), dtype=np.float32).astype(bf)
    return [
        {"x": xb[c * B:(c + 1) * B], "xT": xT[c * B:(c + 1) * B],
         "w1t": w1t, "wo1t": wo1t, "w2t": w2t, "wo2t": wo2t,
         "ident": ident, "onesc": onesc, "onesr": onesr}
        for c in range(NCORES)
    ]


def run(x, W1, Wo1, W2, Wo2, trace=False, **kw):
    from concourse.bass_utils import run_bass_kernel_spmd
    nc = get_nc()
    in_maps = make_in_maps(x, W1, Wo1, W2, Wo2)
    res = run_bass_kernel_spmd(nc, in_maps, core_ids=list(range(NCORES)),
                               trace=trace, **kw)
    out = np.concatenate([res.results[c]["out"] for c in range(NCORES)], axis=0)
    return out.reshape(N_GLOBAL, D, 1, 1), res


def kernel(**inputs):
    out, _ = run(inputs["x"], inputs["W1"], inputs["Wo1"],
                 inputs["W2"], inputs["Wo2"])
    return out


# revision 77
# speedup vs baseline: 1.0249x; 1.0249x over previous
"""AttentionFuserV3 Trainium2 kernel: 8-core pure data parallel over batch.

Reference computation per batch item x_b [L=1024, D=512]:
  stage1: q = x W1^T; S = q x^T; A = softmax(S); mix = A x;
          h = tanh([mix, q] Wo1^T); h = h / max(||h||_2, eps)     (per row)
  stage2: c = [h, x]; q2 = c W2^T; S2 = q2 c^T; A2 = softmax(S2);
          mix2 = A2 c; o = [mix2, q2] Wo2^T; emb = mean_l(o)

Pooling algebra: emb = mean_l(o) is linear, so the full [L,2D] mix2 and
[L,D] output projection are never materialized.  Instead
  emb = [colsum(A2) c, colsum(q2)] (Wo2^T / L)
where colsum(A2)[m] = sum_l exp(S2[l,m]) / denom[l] is a cheap
multiply+reduce over the already-computed exp tiles.  This removes the
two largest matmul groups of stage 2.

Layout strategy ("T-space"): all big tensors are kept transposed in SBUF
(feature dim on partitions, sequence dim L on the free axis) so every
matmul contraction lands on the partition axis.  Softmax runs without
max-subtraction (|scores| < ~70, exp stays in range); the denominator is
accumulated with a ones-vector matmul and applied as a column broadcast
produced by a rank-1 matmul.

All matmul operands are bf16 (same 1 cycle/row PE speed as f32r, half
the SBUF/DMA); accumulation stays in f32 PSUM.  The halved SBUF lets
every per-batch tile be double-buffered, and the program is emitted as
an explicit software pipeline: stage 2 of batch b is interleaved with
stage 1 of batch b+1 at phase granularity (and stage-1 phases alternate
their two l-chunks) so the in-order PE queue always has independent
matmuls between a producer phase and its consumer.
"""

import sys

sys.path.insert(0, "/opt/trn_rl_repo")

import numpy as np

N_GLOBAL, L, D = 32, 1024, 512
NCORES = 8
B = N_GLOBAL // NCORES          # 4 batch items per core
P = 128
LC = 512                        # l-chunk (matmul moving free dim)
NLC = L // LC                   # 2
DT = D // P                     # 4
LT = L // P                     # 8
D2T = 2 * D // P                # 8
C2T = 4 * D // P                # 16

_CACHE = {}


def _build_nc():
    import concourse.bass as bass  # noqa: F401
    import concourse.mybir as mybir
    import concourse.tile as tile
    from concourse import bacc

    f32 = mybir.dt.float32
    bf16 = mybir.dt.bfloat16
    AF = mybir.ActivationFunctionType
    ALU = mybir.AluOpType

    nc = bacc.Bacc("TRN2", target_bir_lowering=False, debug=False,
                   num_devices=NCORES)

    x_ext = nc.declare_dram_parameter("x", [B, L, D], bf16, isOutput=False)
    xT_ext = nc.declare_dram_parameter("xT", [B, D, L], bf16, isOutput=False)
    w1t_ext = nc.declare_dram_parameter("w1t", [D, D], bf16, isOutput=False)
    wo1t_ext = nc.declare_dram_parameter("wo1t", [2 * D, D], bf16, isOutput=False)
    w2t_ext = nc.declare_dram_parameter("w2t", [2 * D, 2 * D], bf16, isOutput=False)
    wo2t_ext = nc.declare_dram_parameter("wo2t", [4 * D, D], bf16, isOutput=False)
    id_ext = nc.declare_dram_parameter("ident", [P, P], bf16, isOutput=False)
    onc_ext = nc.declare_dram_parameter("onesc", [P, 1], bf16, isOutput=False)
    onr_ext = nc.declare_dram_parameter("onesr", [1, P], bf16, isOutput=False)
    out_ext = nc.declare_dram_parameter("out", [B, D], f32, isOutput=True)

    import time as _time
    _t0 = _time.time()
    with tile.TileContext(nc) as tc:
        with tc.tile_pool(name="wp", bufs=1) as wp, \
             tc.tile_pool(name="cp", bufs=1) as cp, \
             tc.tile_pool(name="xp", bufs=2) as xp, \
             tc.tile_pool(name="hp", bufs=2) as hp, \
             tc.tile_pool(name="tp", bufs=2) as tp, \
             tc.tile_pool(name="vp", bufs=2) as vp, \
             tc.tile_pool(name="ps", bufs=8, space="PSUM") as pp:

            # ---- per-batch tile state (input DMAs issued before the bulky
            # weight DMAs so ph1 of batch 0 can start early) ----
            T = [dict() for _ in range(B)]

            def start_batch(b, defer_x=False):
                t = T[b]
                t["xT"] = xp.tile([P, DT, L], bf16, tag="xT", name=f"xT_{b}")
                nc.sync.dma_start(out=t["xT"],
                                  in_=xT_ext[b].rearrange("(k p) l -> p k l", p=P))
                if not defer_x:
                    start_x(b)
                t["hTn"] = hp.tile([P, DT, L], bf16, tag="hTn", name=f"hTn_{b}")
                t["qT"] = [None] * NLC
                t["mixT"] = [None] * NLC
                t["expT"] = [None] * NLC
                t["hT"] = [None] * NLC
                t["q2T"] = [None] * NLC
                t["exp2"] = [None] * NLC
                t["rv1"] = [None] * NLC
                t["rv2"] = [None] * NLC
                t["rv3"] = [None] * NLC
                t["a2sr"] = [None] * NLC

            def start_x(b):
                t = T[b]
                t["x"] = xp.tile([P, LT, D], bf16, tag="x", name=f"x_{b}")
                nc.sync.dma_start(out=t["x"],
                                  in_=x_ext[b].rearrange("(k p) d -> p k d", p=P))

            start_batch(0, defer_x=True)

            # ---- weights + constants (resident) ----
            w1t_s = wp.tile([P, DT, D], bf16, tag="w1t")
            nc.sync.dma_start(out=w1t_s, in_=w1t_ext.rearrange("(k p) e -> p k e", p=P))
            ones_s = cp.tile([P, 1], bf16, tag="ones")
            nc.sync.dma_start(out=ones_s, in_=onc_ext[:, :])
            onesr_s = cp.tile([1, P], bf16, tag="onesr")
            nc.sync.dma_start(out=onesr_s, in_=onr_ext[:, :])
            start_x(0)
            wo1t_s = wp.tile([P, D2T, D], bf16, tag="wo1t")
            nc.sync.dma_start(out=wo1t_s, in_=wo1t_ext.rearrange("(k p) e -> p k e", p=P))
            ident_s = cp.tile([P, P], bf16, tag="ident")
            nc.sync.dma_start(out=ident_s, in_=id_ext[:, :])
            w2t_s = wp.tile([P, D2T, 2 * D], bf16, tag="w2t")
            nc.sync.dma_start(out=w2t_s, in_=w2t_ext.rearrange("(k p) e -> p k e", p=P))
            wo2t_s = wp.tile([P, C2T, D], bf16, tag="wo2t")
            nc.sync.dma_start(out=wo2t_s, in_=wo2t_ext.rearrange("(k p) e -> p k e", p=P))

            def mm(out, lhsT, rhs, first, last):
                nc.tensor.matmul(out, lhsT, rhs, start=first, stop=last)

            def recip_part(denom_ps, nm, clamp_eps=None):
                """[1,512] PSUM denominator -> [1,512] SBUF reciprocal
                (optionally sqrt+clamp first)."""
                rv = vp.tile([1, LC], bf16, tag="rv", bufs=3, name=f"rv_{nm}")
                with nc.allow_low_precision(reason="bf16 softmax scale"):
                    if clamp_eps is not None:
                        nv = vp.tile([1, LC], f32, tag="nv", bufs=1,
                                     name=f"nv_{nm}")
                        nc.scalar.sqrt(nv, denom_ps[0:1, :])
                        nc.vector.tensor_scalar_max(nv, nv, clamp_eps)
                        nc.vector.reciprocal(rv, nv)
                    else:
                        nc.vector.reciprocal(rv, denom_ps[0:1, :])
                return rv

            def bcast_part(rv, nm):
                """[1,512] reciprocal -> [128,512] broadcast via rank-1 matmul.
                Emitted a block after recip_part so the PE never waits on the
                DVE reciprocal."""
                ps_b = pp.tile([P, LC], f32, tag="ps", name=f"psb_{nm}")
                mm(ps_b, onesr_s, rv[0:1, :], True, True)
                bc = vp.tile([P, LC], f32, tag="bc", bufs=3, name=f"bc_{nm}")
                nc.scalar.copy(bc, ps_b)
                return bc

            # ================= stage 1 phases =================
            def ph1(b, lc):
                t = T[b]
                ls = slice(lc * LC, (lc + 1) * LC)
                qT = tp.tile([P, DT, LC], bf16, tag="qt", name=f"qT_{b}_{lc}")
                t["qT"][lc] = qT
                for et in range(DT):
                    ps = pp.tile([P, LC], f32, tag="ps", name=f"ps1_{b}_{lc}_{et}")
                    for dk in range(DT):
                        mm(ps, w1t_s[:, dk, et * P:(et + 1) * P],
                           t["xT"][:, dk, ls], dk == 0, dk == DT - 1)
                    nc.scalar.copy(qT[:, et, :], ps)

            def ph2(b, lc):
                t = T[b]
                expT = tp.tile([P, LT, LC], bf16, tag="exp", bufs=3,
                               name=f"expT_{b}_{lc}")
                t["expT"][lc] = expT
                ps_d = pp.tile([P, LC], f32, tag="ps", name=f"psd1_{b}_{lc}")
                for mt in range(LT):
                    ps = pp.tile([P, LC], f32, tag="ps", name=f"ps2_{b}_{lc}_{mt}")
                    for ek in range(DT):
                        mm(ps, t["xT"][:, ek, mt * P:(mt + 1) * P],
                           t["qT"][lc][:, ek, :], ek == 0, ek == DT - 1)
                    nc.scalar.activation(expT[:, mt, :], ps, AF.Exp)
                    mm(ps_d[0:1, :], ones_s, expT[:, mt, :],
                       mt == 0, mt == LT - 1)
                t["rv1"][lc] = recip_part(ps_d, f"b1_{b}_{lc}")

            def ph3(b, lc):
                t = T[b]
                bc1 = bcast_part(t["rv1"][lc], f"b1_{b}_{lc}")
                mixT = tp.tile([P, DT, LC], bf16, tag="mix", name=f"mixT_{b}_{lc}")
                t["mixT"][lc] = mixT
                ps_m = [pp.tile([P, LC], f32, tag="ps", name=f"psm_{b}_{lc}_{i}")
                        for i in range(DT)]
                for mk in range(LT):
                    for dt in range(DT):
                        mm(ps_m[dt], t["x"][:, mk, dt * P:(dt + 1) * P],
                           t["expT"][lc][:, mk, :], mk == 0, mk == LT - 1)
                for dt in range(DT):
                    nc.vector.tensor_mul(mixT[:, dt, :], ps_m[dt], bc1)

            def ph4(b, lc):
                t = T[b]
                hT = tp.tile([P, DT, LC], bf16, tag="ht", name=f"hT_{b}_{lc}")
                t["hT"][lc] = hT
                for ot in range(DT):
                    ps = pp.tile([P, LC], f32, tag="ps", name=f"ps4_{b}_{lc}_{ot}")
                    for ck in range(D2T):
                        rhs = (t["mixT"][lc][:, ck, :] if ck < DT
                               else t["qT"][lc][:, ck - DT, :])
                        mm(ps, wo1t_s[:, ck, ot * P:(ot + 1) * P],
                           rhs, ck == 0, ck == D2T - 1)
                    nc.scalar.activation(hT[:, ot, :], ps, AF.Tanh)

            def ph5a(b, lc):
                t = T[b]
                hsq = tp.tile([P, DT, LC], bf16, tag="hsq", name=f"hsq_{b}_{lc}")
                for dt in range(DT):
                    nc.scalar.activation(hsq[:, dt, :], t["hT"][lc][:, dt, :],
                                         AF.Square)
                ps_n = pp.tile([P, LC], f32, tag="ps", name=f"psn_{b}_{lc}")
                for dt in range(DT):
                    mm(ps_n[0:1, :], ones_s, hsq[:, dt, :], dt == 0, dt == DT - 1)
                t["rv2"][lc] = recip_part(ps_n, f"b2_{b}_{lc}", clamp_eps=1e-12)

            def ph5b(b, lc):
                t = T[b]
                ls = slice(lc * LC, (lc + 1) * LC)
                bc2 = bcast_part(t["rv2"][lc], f"b2_{b}_{lc}")
                for dt in range(DT):
                    nc.vector.tensor_mul(t["hTn"][:, dt, ls], t["hT"][lc][:, dt, :],
                                         bc2)

            # ================= stage 2 phases =================
            def c2T(t, k, fslice):
                """combined2T[d2,·] k-tile: [hTn; xT]"""
                return (t["hTn"][:, k, fslice] if k < DT
                        else t["xT"][:, k - DT, fslice])

            def ph7(b, lc):
                t = T[b]
                ls = slice(lc * LC, (lc + 1) * LC)
                if lc == 0:
                    t["q2r"] = vp.tile([P, D2T, NLC], f32, tag="q2r", bufs=2,
                                       name=f"q2r_{b}")
                q2T = tp.tile([P, D2T, LC], bf16, tag="q2", name=f"q2T_{b}_{lc}")
                t["q2T"][lc] = q2T
                for et in range(D2T):
                    ps = pp.tile([P, LC], f32, tag="ps", name=f"ps7_{b}_{lc}_{et}")
                    for dk in range(D2T):
                        mm(ps, w2t_s[:, dk, et * P:(et + 1) * P],
                           c2T(t, dk, ls), dk == 0, dk == D2T - 1)
                    nc.scalar.copy(q2T[:, et, :], ps)
                    nc.vector.tensor_reduce(t["q2r"][:, et, lc:lc + 1], ps,
                                            axis=mybir.AxisListType.X,
                                            op=ALU.add)

            def ph8a(b, lc):
                t = T[b]
                if lc == 0:
                    t["a2p"] = vp.tile([P, LT, NLC], f32, tag="a2p", bufs=2,
                                       name=f"a2p_{b}")
                    t["scr"] = vp.tile([P, LC], f32, tag="scr", bufs=1,
                                       name=f"scr_{b}")
                exp2 = tp.tile([P, LT, LC], bf16, tag="exp", bufs=3,
                                name=f"exp2_{b}_{lc}")
                t["exp2"][lc] = exp2
                ps_d = pp.tile([P, LC], f32, tag="ps", name=f"psd2_{b}_{lc}")
                for mt in range(LT):
                    ps = pp.tile([P, LC], f32, tag="ps", name=f"ps8_{b}_{lc}_{mt}")
                    for ek in range(D2T):
                        mm(ps, c2T(t, ek, slice(mt * P, (mt + 1) * P)),
                           t["q2T"][lc][:, ek, :], ek == 0, ek == D2T - 1)
                    nc.scalar.activation(exp2[:, mt, :], ps, AF.Exp)
                    mm(ps_d[0:1, :], ones_s, exp2[:, mt, :],
                       mt == 0, mt == LT - 1)
                t["rv3"][lc] = recip_part(ps_d, f"b3_{b}_{lc}")

            def ph8b(b, lc):
                # A2 column sums a2p[m,lc] = sum_{l in chunk} exp2[m,l]/denom[l]
                # via fused multiply+reduce on the DVE
                t = T[b]
                bc3 = bcast_part(t["rv3"][lc], f"b3_{b}_{lc}")
                for mt in range(LT):
                    nc.vector.affine_mul_reduce(
                        out=t["scr"], accum_out=t["a2p"][:, mt, lc:lc + 1],
                        in0=t["exp2"][lc][:, mt, :], in1=bc3,
                        scale=1.0, bias=0.0)

            def epi_mid(b):
                # comb = [colsum(A2) @ c, colsum(q2)] as a [4D] column vector.
                # colsum(A2) is transposed to a row (tiny PE transposes),
                # broadcast over partitions (rank-1 matmuls), then contracted
                # against the T-space tiles of c = [hn, x] with fused
                # multiply+reduce -- no natural-layout hidden, no DRAM bounce.
                t = T[b]
                a2s = vp.tile([P, LT], f32, tag="a2s", bufs=2, name=f"a2s_{b}")
                nc.vector.tensor_add(a2s, t["a2p"][:, :, 0], t["a2p"][:, :, 1])
                a2sc = vp.tile([P, LT], bf16, tag="a2sc", bufs=2,
                               name=f"a2sc_{b}")
                nc.scalar.copy(a2sc, a2s)
                a2row = vp.tile([1, L], bf16, tag="a2row", bufs=2,
                                name=f"a2row_{b}")
                for mt in range(LT):
                    ps_t = pp.tile([1, P], bf16, tag="ps", name=f"pst_{b}_{mt}")
                    nc.tensor.transpose(ps_t, a2sc[:, mt:mt + 1], ident_s)
                    nc.scalar.copy(a2row[0:1, mt * P:(mt + 1) * P], ps_t)
                a2b = vp.tile([P, L], bf16, tag="a2b", bufs=2, name=f"a2b_{b}")
                for lc in range(NLC):
                    ls = slice(lc * LC, (lc + 1) * LC)
                    ps_bc = pp.tile([P, LC], f32, tag="ps",
                                    name=f"psbc_{b}_{lc}")
                    mm(ps_bc, onesr_s, a2row[0:1, ls], True, True)
                    nc.scalar.copy(a2b[:, ls], ps_bc)
                comb = vp.tile([P, C2T], f32, tag="comb", bufs=2, name=f"comb_{b}")
                t["comb"] = comb
                nc.vector.tensor_add(comb[:, D2T:C2T], t["q2r"][:, :, 0],
                                     t["q2r"][:, :, 1])
                scrL = vp.tile([P, L], bf16, tag="scrL", bufs=1,
                               name=f"scrL_{b}")
                for dt in range(DT):
                    nc.vector.affine_mul_reduce(
                        out=scrL, accum_out=comb[:, dt:dt + 1],
                        in0=t["hTn"][:, dt, :], in1=a2b, scale=1.0, bias=0.0)
                for dt in range(DT):
                    nc.vector.affine_mul_reduce(
                        out=scrL, accum_out=comb[:, DT + dt:DT + dt + 1],
                        in0=t["xT"][:, dt, :], in1=a2b, scale=1.0, bias=0.0)

            def epiB(b):
                # emb = comb @ (Wo2^T/L) as a single [1,D] row (1/L folded
                # into wo2t on the host)
                t = T[b]
                combr = vp.tile([P, C2T], bf16, tag="combr", bufs=2,
                                name=f"combr_{b}")
                nc.scalar.copy(combr, t["comb"])
                ps_o = pp.tile([1, D], f32, tag="ps", name=f"pso_{b}")
                for ck in range(C2T):
                    mm(ps_o[0:1, :], combr[:, ck:ck + 1], wo2t_s[:, ck, :],
                       ck == 0, ck == C2T - 1)
                orow = vp.tile([1, D], f32, tag="orow", bufs=2, name=f"orow_{b}")
                nc.scalar.copy(orow, ps_o)
                nc.sync.dma_start(out=out_ext[b:b + 1, :], in_=orow[0:1, :])

            # ================= emission schedule =================
            # Software pipeline: stage 2 of batch b (A-blocks) interleaved
            # with stage 1 of batch b+1 (B-blocks); epiB(b) is deferred into
            # iteration b+1 so the DRAM bounce is in flight under ph7.
            def S1(b):
                return [lambda lc=lc, f=f: f(b, lc)
                        for f in (ph1, ph2, ph3) for lc in range(NLC)
                        ] + [lambda: ph4(b, 0), lambda: ph5a(b, 0),
                             lambda: ph4(b, 1), lambda: ph5a(b, 1),
                             lambda: ph5b(b, 0), lambda: ph5b(b, 1)]

            def S2(b):
                return [lambda: ph7(b, 0),
                        lambda: ph8a(b, 0),
                        lambda: (ph7(b, 1), ph8b(b, 0)),
                        lambda: ph8a(b, 1),
                        lambda: ph8b(b, 1),
                        lambda: epi_mid(b)]

            for f in S1(0):
                f()
            for b in range(B):
                A = S2(b)
                A[0]()
                if b + 1 < B:
                    if b > 0:
                        epiB(b - 1)
                    start_batch(b + 1)
                    Bl = S1(b + 1)
                    Bl[0](); Bl[1]()
                    A[1]()
                    Bl[2](); Bl[3]()
                    A[2]()
                    Bl[4](); Bl[5]()
                    A[3]()
                    Bl[6]()
                    A[4]()
                    Bl[7](); Bl[8](); Bl[9](); Bl[10](); Bl[11]()
                    A[5]()
                else:
                    # last batch has no stage-1 filler: spend epiB(b-1)'s
                    # independent matmuls covering the epilogue DVE chain
                    A[1](); A[2](); A[3](); A[4]()
                    epiB(b - 1)
                    A[5]()
            epiB(B - 1)

    _t1 = _time.time()
    nc.compile()
    print(f"[kernel] tile-trace+schedule {_t1 - _t0:.1f}s, "
          f"bacc compile {_time.time() - _t1:.1f}s", file=sys.stderr, flush=True)
    return nc


def get_nc():
    if "nc" not in _CACHE:
        _CACHE["nc"] = _build_nc()
    return _CACHE["nc"]


def make_in_maps(x, W1, Wo1, W2, Wo2):
    import ml_dtypes
    bf = ml_dtypes.bfloat16
    x = np.ascontiguousarray(np.asarray(x, dtype=np.float32))
    xT = np.ascontiguousarray(x.transpose(0, 2, 1)).astype(bf)
    xb = x.astype(bf)
    w1t = np.ascontiguousarray(np.asarray(W1, np.float32).T).astype(bf)
    wo1t = np.ascontiguousarray(np.asarray(Wo1, np.float32).T).astype(bf)
    w2t = np.ascontiguousarray(np.asarray(W2, np.float32).T).astype(bf)
    # 1/L mean-pooling factor folded into the stage-2 output projection
    wo2t = (np.ascontiguousarray(np.asarray(Wo2, np.float32).T)
            * (1.0 / L)).astype(bf)
    ident = np.eye(P, dtype=np.float32).astype(bf)
    onesc = np.ones((P, 1), dtype=np.float32).astype(bf)
    onesr = np.ones((1, P), dtype=np.float32).astype(bf)
    return [
        {"x": xb[c * B:(c + 1) * B], "xT": xT[c * B:(c + 1) * B],
         "w1t": w1t, "wo1t": wo1t, "w2t": w2t, "wo2t": wo2t,
         "ident": ident, "onesc": onesc, "onesr": onesr}
        for c in range(NCORES)
    ]


def run(x, W1, Wo1, W2, Wo2, trace=False, **kw):
    from concourse.bass_utils import run_bass_kernel_spmd
    nc = get_nc()
    in_maps = make_in_maps(x, W1, Wo1, W2, Wo2)
    res = run_bass_kernel_spmd(nc, in_maps, core_ids=list(range(NCORES)),
                               trace=trace, **kw)
    out = np.concatenate([res.results[c]["out"] for c in range(NCORES)], axis=0)
    return out.reshape(N_GLOBAL, D, 1, 1), res


def kernel(**inputs):
    out, _ = run(inputs["x"], inputs["W1"], inputs["Wo1"],
                 inputs["W2"], inputs["Wo2"])
    return out


# revision 78
# speedup vs baseline: 1.0267x; 1.0017x over previous
"""AttentionFuserV3 Trainium2 kernel: 8-core pure data parallel over batch.

Reference computation per batch item x_b [L=1024, D=512]:
  stage1: q = x W1^T; S = q x^T; A = softmax(S); mix = A x;
          h = tanh([mix, q] Wo1^T); h = h / max(||h||_2, eps)     (per row)
  stage2: c = [h, x]; q2 = c W2^T; S2 = q2 c^T; A2 = softmax(S2);
          mix2 = A2 c; o = [mix2, q2] Wo2^T; emb = mean_l(o)

Pooling algebra: emb = mean_l(o) is linear, so the full [L,2D] mix2 and
[L,D] output projection are never materialized.  Instead
  emb = [colsum(A2) c, colsum(q2)] (Wo2^T / L)
where colsum(A2)[m] = sum_l exp(S2[l,m]) / denom[l] is a cheap
multiply+reduce over the already-computed exp tiles.  This removes the
two largest matmul groups of stage 2.

Layout strategy ("T-space"): all big tensors are kept transposed in SBUF
(feature dim on partitions, sequence dim L on the free axis) so every
matmul contraction lands on the partition axis.  Softmax runs without
max-subtraction (|scores| < ~70, exp stays in range); the denominator is
accumulated with a ones-vector matmul and applied as a column broadcast
produced by a rank-1 matmul.

All matmul operands are bf16 (same 1 cycle/row PE speed as f32r, half
the SBUF/DMA); accumulation stays in f32 PSUM.  The halved SBUF lets
every per-batch tile be double-buffered, and the program is emitted as
an explicit software pipeline: stage 2 of batch b is interleaved with
stage 1 of batch b+1 at phase granularity (and stage-1 phases alternate
their two l-chunks) so the in-order PE queue always has independent
matmuls between a producer phase and its consumer.
"""

import sys

sys.path.insert(0, "/opt/trn_rl_repo")

import numpy as np

N_GLOBAL, L, D = 32, 1024, 512
NCORES = 8
B = N_GLOBAL // NCORES          # 4 batch items per core
P = 128
LC = 512                        # l-chunk (matmul moving free dim)
NLC = L // LC                   # 2
DT = D // P                     # 4
LT = L // P                     # 8
D2T = 2 * D // P                # 8
C2T = 4 * D // P                # 16

_CACHE = {}


def _build_nc():
    import concourse.bass as bass  # noqa: F401
    import concourse.mybir as mybir
    import concourse.tile as tile
    from concourse import bacc

    f32 = mybir.dt.float32
    bf16 = mybir.dt.bfloat16
    AF = mybir.ActivationFunctionType
    ALU = mybir.AluOpType

    nc = bacc.Bacc("TRN2", target_bir_lowering=False, debug=False,
                   num_devices=NCORES)

    x_ext = nc.declare_dram_parameter("x", [B, L, D], bf16, isOutput=False)
    xT_ext = nc.declare_dram_parameter("xT", [B, D, L], bf16, isOutput=False)
    w1t_ext = nc.declare_dram_parameter("w1t", [D, D], bf16, isOutput=False)
    wo1t_ext = nc.declare_dram_parameter("wo1t", [2 * D, D], bf16, isOutput=False)
    w2t_ext = nc.declare_dram_parameter("w2t", [2 * D, 2 * D], bf16, isOutput=False)
    wo2t_ext = nc.declare_dram_parameter("wo2t", [4 * D, D], bf16, isOutput=False)
    id_ext = nc.declare_dram_parameter("ident", [P, P], bf16, isOutput=False)
    onc_ext = nc.declare_dram_parameter("onesc", [P, 1], bf16, isOutput=False)
    onr_ext = nc.declare_dram_parameter("onesr", [1, P], bf16, isOutput=False)
    out_ext = nc.declare_dram_parameter("out", [B, D], f32, isOutput=True)

    import time as _time
    _t0 = _time.time()
    with tile.TileContext(nc) as tc:
        with tc.tile_pool(name="wp", bufs=1) as wp, \
             tc.tile_pool(name="cp", bufs=1) as cp, \
             tc.tile_pool(name="xp", bufs=2) as xp, \
             tc.tile_pool(name="hp", bufs=2) as hp, \
             tc.tile_pool(name="tp", bufs=2) as tp, \
             tc.tile_pool(name="vp", bufs=2) as vp, \
             tc.tile_pool(name="ps", bufs=8, space="PSUM") as pp:

            # ---- per-batch tile state (input DMAs issued before the bulky
            # weight DMAs so ph1 of batch 0 can start early) ----
            T = [dict() for _ in range(B)]

            def start_batch(b, defer_x=False):
                t = T[b]
                t["xT"] = xp.tile([P, DT, L], bf16, tag="xT", name=f"xT_{b}")
                nc.sync.dma_start(out=t["xT"],
                                  in_=xT_ext[b].rearrange("(k p) l -> p k l", p=P))
                if not defer_x:
                    start_x(b)
                t["hTn"] = hp.tile([P, DT, L], bf16, tag="hTn", name=f"hTn_{b}")
                t["qT"] = [None] * NLC
                t["mixT"] = [None] * NLC
                t["expT"] = [None] * NLC
                t["hT"] = [None] * NLC
                t["q2T"] = [None] * NLC
                t["exp2"] = [None] * NLC
                t["rv1"] = [None] * NLC
                t["rv2"] = [None] * NLC
                t["rv3"] = [None] * NLC
                t["a2sr"] = [None] * NLC

            def start_x(b):
                t = T[b]
                t["x"] = xp.tile([P, LT, D], bf16, tag="x", name=f"x_{b}")
                nc.sync.dma_start(out=t["x"],
                                  in_=x_ext[b].rearrange("(k p) d -> p k d", p=P))

            start_batch(0, defer_x=True)

            # ---- weights + constants (resident) ----
            w1t_s = wp.tile([P, DT, D], bf16, tag="w1t")
            nc.sync.dma_start(out=w1t_s, in_=w1t_ext.rearrange("(k p) e -> p k e", p=P))
            ones_s = cp.tile([P, 1], bf16, tag="ones")
            nc.sync.dma_start(out=ones_s, in_=onc_ext[:, :])
            onesr_s = cp.tile([1, P], bf16, tag="onesr")
            nc.sync.dma_start(out=onesr_s, in_=onr_ext[:, :])
            start_x(0)
            wo1t_s = wp.tile([P, D2T, D], bf16, tag="wo1t")
            nc.sync.dma_start(out=wo1t_s, in_=wo1t_ext.rearrange("(k p) e -> p k e", p=P))
            ident_s = cp.tile([P, P], bf16, tag="ident")
            nc.sync.dma_start(out=ident_s, in_=id_ext[:, :])
            w2t_s = wp.tile([P, D2T, 2 * D], bf16, tag="w2t")
            nc.sync.dma_start(out=w2t_s, in_=w2t_ext.rearrange("(k p) e -> p k e", p=P))
            wo2t_s = wp.tile([P, C2T, D], bf16, tag="wo2t")
            nc.sync.dma_start(out=wo2t_s, in_=wo2t_ext.rearrange("(k p) e -> p k e", p=P))

            def mm(out, lhsT, rhs, first, last):
                nc.tensor.matmul(out, lhsT, rhs, start=first, stop=last)

            def recip_part(denom_ps, nm, clamp_eps=None):
                """[1,512] PSUM denominator -> [1,512] SBUF reciprocal
                (optionally sqrt+clamp first)."""
                rv = vp.tile([1, LC], bf16, tag="rv", bufs=3, name=f"rv_{nm}")
                with nc.allow_low_precision(reason="bf16 softmax scale"):
                    if clamp_eps is not None:
                        nv = vp.tile([1, LC], f32, tag="nv", bufs=1,
                                     name=f"nv_{nm}")
                        nc.scalar.sqrt(nv, denom_ps[0:1, :])
                        nc.vector.tensor_scalar_max(nv, nv, clamp_eps)
                        nc.vector.reciprocal(rv, nv)
                    else:
                        nc.vector.reciprocal(rv, denom_ps[0:1, :])
                return rv

            def bcast_part(rv, nm):
                """[1,512] reciprocal -> [128,512] broadcast via rank-1 matmul.
                Emitted a block after recip_part so the PE never waits on the
                DVE reciprocal."""
                ps_b = pp.tile([P, LC], f32, tag="ps", name=f"psb_{nm}")
                mm(ps_b, onesr_s, rv[0:1, :], True, True)
                bc = vp.tile([P, LC], bf16, tag="bc", bufs=3, name=f"bc_{nm}")
                nc.scalar.copy(bc, ps_b)
                return bc

            # ================= stage 1 phases =================
            def ph1(b, lc):
                t = T[b]
                ls = slice(lc * LC, (lc + 1) * LC)
                qT = tp.tile([P, DT, LC], bf16, tag="qt", name=f"qT_{b}_{lc}")
                t["qT"][lc] = qT
                for et in range(DT):
                    ps = pp.tile([P, LC], f32, tag="ps", name=f"ps1_{b}_{lc}_{et}")
                    for dk in range(DT):
                        mm(ps, w1t_s[:, dk, et * P:(et + 1) * P],
                           t["xT"][:, dk, ls], dk == 0, dk == DT - 1)
                    nc.scalar.copy(qT[:, et, :], ps)

            def ph2(b, lc):
                t = T[b]
                expT = tp.tile([P, LT, LC], bf16, tag="exp", bufs=3,
                               name=f"expT_{b}_{lc}")
                t["expT"][lc] = expT
                ps_d = pp.tile([P, LC], f32, tag="ps", name=f"psd1_{b}_{lc}")
                for mt in range(LT):
                    ps = pp.tile([P, LC], f32, tag="ps", name=f"ps2_{b}_{lc}_{mt}")
                    for ek in range(DT):
                        mm(ps, t["xT"][:, ek, mt * P:(mt + 1) * P],
                           t["qT"][lc][:, ek, :], ek == 0, ek == DT - 1)
                    nc.scalar.activation(expT[:, mt, :], ps, AF.Exp)
                    mm(ps_d[0:1, :], ones_s, expT[:, mt, :],
                       mt == 0, mt == LT - 1)
                t["rv1"][lc] = recip_part(ps_d, f"b1_{b}_{lc}")

            def ph3(b, lc):
                t = T[b]
                bc1 = bcast_part(t["rv1"][lc], f"b1_{b}_{lc}")
                mixT = tp.tile([P, DT, LC], bf16, tag="mix", name=f"mixT_{b}_{lc}")
                t["mixT"][lc] = mixT
                ps_m = [pp.tile([P, LC], f32, tag="ps", name=f"psm_{b}_{lc}_{i}")
                        for i in range(DT)]
                for mk in range(LT):
                    for dt in range(DT):
                        mm(ps_m[dt], t["x"][:, mk, dt * P:(dt + 1) * P],
                           t["expT"][lc][:, mk, :], mk == 0, mk == LT - 1)
                for dt in range(DT):
                    nc.vector.tensor_mul(mixT[:, dt, :], ps_m[dt], bc1)

            def ph4(b, lc):
                t = T[b]
                hT = tp.tile([P, DT, LC], bf16, tag="ht", name=f"hT_{b}_{lc}")
                t["hT"][lc] = hT
                for ot in range(DT):
                    ps = pp.tile([P, LC], f32, tag="ps", name=f"ps4_{b}_{lc}_{ot}")
                    for ck in range(D2T):
                        rhs = (t["mixT"][lc][:, ck, :] if ck < DT
                               else t["qT"][lc][:, ck - DT, :])
                        mm(ps, wo1t_s[:, ck, ot * P:(ot + 1) * P],
                           rhs, ck == 0, ck == D2T - 1)
                    nc.scalar.activation(hT[:, ot, :], ps, AF.Tanh)

            def ph5a(b, lc):
                t = T[b]
                hsq = tp.tile([P, DT, LC], bf16, tag="hsq", name=f"hsq_{b}_{lc}")
                for dt in range(DT):
                    nc.scalar.activation(hsq[:, dt, :], t["hT"][lc][:, dt, :],
                                         AF.Square)
                ps_n = pp.tile([P, LC], f32, tag="ps", name=f"psn_{b}_{lc}")
                for dt in range(DT):
                    mm(ps_n[0:1, :], ones_s, hsq[:, dt, :], dt == 0, dt == DT - 1)
                t["rv2"][lc] = recip_part(ps_n, f"b2_{b}_{lc}", clamp_eps=1e-12)

            def ph5b(b, lc):
                t = T[b]
                ls = slice(lc * LC, (lc + 1) * LC)
                bc2 = bcast_part(t["rv2"][lc], f"b2_{b}_{lc}")
                for dt in range(DT):
                    nc.vector.tensor_mul(t["hTn"][:, dt, ls], t["hT"][lc][:, dt, :],
                                         bc2)

            # ================= stage 2 phases =================
            def c2T(t, k, fslice):
                """combined2T[d2,·] k-tile: [hTn; xT]"""
                return (t["hTn"][:, k, fslice] if k < DT
                        else t["xT"][:, k - DT, fslice])

            def ph7(b, lc):
                t = T[b]
                ls = slice(lc * LC, (lc + 1) * LC)
                if lc == 0:
                    t["q2r"] = vp.tile([P, D2T, NLC], f32, tag="q2r", bufs=2,
                                       name=f"q2r_{b}")
                q2T = tp.tile([P, D2T, LC], bf16, tag="q2", name=f"q2T_{b}_{lc}")
                t["q2T"][lc] = q2T
                for et in range(D2T):
                    ps = pp.tile([P, LC], f32, tag="ps", name=f"ps7_{b}_{lc}_{et}")
                    for dk in range(D2T):
                        mm(ps, w2t_s[:, dk, et * P:(et + 1) * P],
                           c2T(t, dk, ls), dk == 0, dk == D2T - 1)
                    nc.scalar.copy(q2T[:, et, :], ps)
                    nc.vector.tensor_reduce(t["q2r"][:, et, lc:lc + 1], ps,
                                            axis=mybir.AxisListType.X,
                                            op=ALU.add)

            def ph8a(b, lc):
                t = T[b]
                if lc == 0:
                    t["a2p"] = vp.tile([P, LT, NLC], f32, tag="a2p", bufs=2,
                                       name=f"a2p_{b}")
                    t["scr"] = vp.tile([P, LC], bf16, tag="scr", bufs=1,
                                       name=f"scr_{b}")
                exp2 = tp.tile([P, LT, LC], bf16, tag="exp", bufs=3,
                                name=f"exp2_{b}_{lc}")
                t["exp2"][lc] = exp2
                ps_d = pp.tile([P, LC], f32, tag="ps", name=f"psd2_{b}_{lc}")
                for mt in range(LT):
                    ps = pp.tile([P, LC], f32, tag="ps", name=f"ps8_{b}_{lc}_{mt}")
                    for ek in range(D2T):
                        mm(ps, c2T(t, ek, slice(mt * P, (mt + 1) * P)),
                           t["q2T"][lc][:, ek, :], ek == 0, ek == D2T - 1)
                    nc.scalar.activation(exp2[:, mt, :], ps, AF.Exp)
                    mm(ps_d[0:1, :], ones_s, exp2[:, mt, :],
                       mt == 0, mt == LT - 1)
                t["rv3"][lc] = recip_part(ps_d, f"b3_{b}_{lc}")

            def ph8b(b, lc):
                # A2 column sums a2p[m,lc] = sum_{l in chunk} exp2[m,l]/denom[l]
                # via fused multiply+reduce on the DVE
                t = T[b]
                bc3 = bcast_part(t["rv3"][lc], f"b3_{b}_{lc}")
                for mt in range(LT):
                    nc.vector.affine_mul_reduce(
                        out=t["scr"], accum_out=t["a2p"][:, mt, lc:lc + 1],
                        in0=t["exp2"][lc][:, mt, :], in1=bc3,
                        scale=1.0, bias=0.0)

            def epi_mid(b):
                # comb = [colsum(A2) @ c, colsum(q2)] as a [4D] column vector.
                # colsum(A2) is transposed to a row (tiny PE transposes),
                # broadcast over partitions (rank-1 matmuls), then contracted
                # against the T-space tiles of c = [hn, x] with fused
                # multiply+reduce -- no natural-layout hidden, no DRAM bounce.
                t = T[b]
                a2s = vp.tile([P, LT], f32, tag="a2s", bufs=2, name=f"a2s_{b}")
                nc.vector.tensor_add(a2s, t["a2p"][:, :, 0], t["a2p"][:, :, 1])
                a2sc = vp.tile([P, LT], bf16, tag="a2sc", bufs=2,
                               name=f"a2sc_{b}")
                nc.scalar.copy(a2sc, a2s)
                a2row = vp.tile([1, L], bf16, tag="a2row", bufs=2,
                                name=f"a2row_{b}")
                for mt in range(LT):
                    ps_t = pp.tile([1, P], bf16, tag="ps", name=f"pst_{b}_{mt}")
                    nc.tensor.transpose(ps_t, a2sc[:, mt:mt + 1], ident_s)
                    nc.scalar.copy(a2row[0:1, mt * P:(mt + 1) * P], ps_t)
                a2b = vp.tile([P, L], bf16, tag="a2b", bufs=2, name=f"a2b_{b}")
                for lc in range(NLC):
                    ls = slice(lc * LC, (lc + 1) * LC)
                    ps_bc = pp.tile([P, LC], f32, tag="ps",
                                    name=f"psbc_{b}_{lc}")
                    mm(ps_bc, onesr_s, a2row[0:1, ls], True, True)
                    nc.scalar.copy(a2b[:, ls], ps_bc)
                comb = vp.tile([P, C2T], f32, tag="comb", bufs=2, name=f"comb_{b}")
                t["comb"] = comb
                nc.vector.tensor_add(comb[:, D2T:C2T], t["q2r"][:, :, 0],
                                     t["q2r"][:, :, 1])
                scrL = vp.tile([P, L], bf16, tag="scrL", bufs=1,
                               name=f"scrL_{b}")
                for dt in range(DT):
                    nc.vector.affine_mul_reduce(
                        out=scrL, accum_out=comb[:, dt:dt + 1],
                        in0=t["hTn"][:, dt, :], in1=a2b, scale=1.0, bias=0.0)
                for dt in range(DT):
                    nc.vector.affine_mul_reduce(
                        out=scrL, accum_out=comb[:, DT + dt:DT + dt + 1],
                        in0=t["xT"][:, dt, :], in1=a2b, scale=1.0, bias=0.0)

            def epiB(b):
                # emb = comb @ (Wo2^T/L) as a single [1,D] row (1/L folded
                # into wo2t on the host)
                t = T[b]
                combr = vp.tile([P, C2T], bf16, tag="combr", bufs=2,
                                name=f"combr_{b}")
                nc.scalar.copy(combr, t["comb"])
                ps_o = pp.tile([1, D], f32, tag="ps", name=f"pso_{b}")
                for ck in range(C2T):
                    mm(ps_o[0:1, :], combr[:, ck:ck + 1], wo2t_s[:, ck, :],
                       ck == 0, ck == C2T - 1)
                orow = vp.tile([1, D], f32, tag="orow", bufs=2, name=f"orow_{b}")
                nc.scalar.copy(orow, ps_o)
                nc.sync.dma_start(out=out_ext[b:b + 1, :], in_=orow[0:1, :])

            # ================= emission schedule =================
            # Software pipeline: stage 2 of batch b (A-blocks) interleaved
            # with stage 1 of batch b+1 (B-blocks); epiB(b) is deferred into
            # iteration b+1 so the DRAM bounce is in flight under ph7.
            def S1(b):
                return [lambda lc=lc, f=f: f(b, lc)
                        for f in (ph1, ph2, ph3) for lc in range(NLC)
                        ] + [lambda: ph4(b, 0), lambda: ph5a(b, 0),
                             lambda: ph4(b, 1), lambda: ph5a(b, 1),
                             lambda: ph5b(b, 0), lambda: ph5b(b, 1)]

            def S2(b):
                return [lambda: ph7(b, 0),
                        lambda: ph8a(b, 0),
                        lambda: (ph7(b, 1), ph8b(b, 0)),
                        lambda: ph8a(b, 1),
                        lambda: ph8b(b, 1),
                        lambda: epi_mid(b)]

            for f in S1(0):
                f()
            for b in range(B):
                A = S2(b)
                A[0]()
                if b + 1 < B:
                    if b > 0:
                        epiB(b - 1)
                    start_batch(b + 1)
                    Bl = S1(b + 1)
                    Bl[0](); Bl[1]()
                    A[1]()
                    Bl[2](); Bl[3]()
                    A[2]()
                    Bl[4](); Bl[5]()
                    A[3]()
                    Bl[6]()
                    A[4]()
                    Bl[7](); Bl[8](); Bl[9](); Bl[10](); Bl[11]()
                    A[5]()
                else:
                    # last batch has no stage-1 filler: spend epiB(b-1)'s
                    # independent matmuls covering the epilogue DVE chain
                    A[1](); A[2](); A[3](); A[4]()
                    epiB(b - 1)
                    A[5]()
            epiB(B - 1)

    _t1 = _time.time()
    nc.compile()
    print(f"[kernel] tile-trace+schedule {_t1 - _t0:.1f}s, "
          f"bacc compile {_time.time() - _t1:.1f}s", file=sys.stderr, flush=True)
    return nc


def get_nc():
    if "nc" not in _CACHE:
        _CACHE["nc"] = _build_nc()
    return _CACHE["nc"]


def make_in_maps(x, W1, Wo1, W2, Wo2):
    import ml_dtypes
    bf = ml_dtypes.bfloat16
    x = np.ascontiguousarray(np.asarray(x, dtype=np.float32))
    xT = np.ascontiguousarray(x.transpose(0, 2, 1)).astype(bf)
    xb = x.astype(bf)
    w1t = np.ascontiguousarray(np.asarray(W1, np.float32).T).astype(bf)
    wo1t = np.ascontiguousarray(np.asarray(Wo1, np.float32).T).astype(bf)
    w2t = np.ascontiguousarray(np.asarray(W2, np.float32).T).astype(bf)
    # 1/L mean-pooling factor folded into the stage-2 output projection
    wo2t = (np.ascontiguousarray(np.asarray(Wo2, np.float32).T)
            * (1.0 / L)).astype(bf)
    ident = np.eye(P, dtype=np.float32).astype(bf)
    onesc = np.ones((P, 1), dtype=np.float32).astype(bf)
    onesr = np.ones((1, P), dtype=np.float32).astype(bf)
    return [
        {"x": xb[c * B:(c + 1) * B], "xT": xT[c * B:(c + 1) * B],
         "w1t": w1t, "wo1t": wo1t, "w2t": w2t, "wo2t": wo2t,
         "ident": ident, "onesc": onesc, "onesr": onesr}
        for c in range(NCORES)
    ]


def run(x, W1, Wo1, W2, Wo2, trace=False, **kw):
    from concourse.bass_utils import run_bass_kernel_spmd
    nc = get_nc()
    in_maps = make_in_maps(x, W1, Wo1, W2, Wo2)
    res = run_bass_kernel_spmd(nc, in_maps, core_ids=list(range(NCORES)),
                               trace=trace, **kw)
    out = np.concatenate([res.results[c]["out"] for c in range(NCORES)], axis=0)
    return out.reshape(N_GLOBAL, D, 1, 1), res


def kernel(**inputs):
    out, _ = run(inputs["x"], inputs["W1"], inputs["Wo1"],
                 inputs["W2"], inputs["Wo2"])
    return out


# revision 79
# speedup vs baseline: 1.0301x; 1.0033x over previous
"""AttentionFuserV3 Trainium2 kernel: 8-core pure data parallel over batch.

Reference computation per batch item x_b [L=1024, D=512]:
  stage1: q = x W1^T; S = q x^T; A = softmax(S); mix = A x;
          h = tanh([mix, q] Wo1^T); h = h / max(||h||_2, eps)     (per row)
  stage2: c = [h, x]; q2 = c W2^T; S2 = q2 c^T; A2 = softmax(S2);
          mix2 = A2 c; o = [mix2, q2] Wo2^T; emb = mean_l(o)

Pooling algebra: emb = mean_l(o) is linear, so the full [L,2D] mix2 and
[L,D] output projection are never materialized.  Instead
  emb = [colsum(A2) c, colsum(q2)] (Wo2^T / L)
where colsum(A2)[m] = sum_l exp(S2[l,m]) / denom[l] is a cheap
multiply+reduce over the already-computed exp tiles.  This removes the
two largest matmul groups of stage 2.

Layout strategy ("T-space"): all big tensors are kept transposed in SBUF
(feature dim on partitions, sequence dim L on the free axis) so every
matmul contraction lands on the partition axis.  Softmax runs without
max-subtraction (|scores| < ~70, exp stays in range); the denominator is
accumulated with a ones-vector matmul and applied as a column broadcast
produced by a rank-1 matmul.

All matmul operands are bf16 (same 1 cycle/row PE speed as f32r, half
the SBUF/DMA); accumulation stays in f32 PSUM.  The halved SBUF lets
every per-batch tile be double-buffered, and the program is emitted as
an explicit software pipeline: stage 2 of batch b is interleaved with
stage 1 of batch b+1 at phase granularity (and stage-1 phases alternate
their two l-chunks) so the in-order PE queue always has independent
matmuls between a producer phase and its consumer.
"""

import sys

sys.path.insert(0, "/opt/trn_rl_repo")

import numpy as np

N_GLOBAL, L, D = 32, 1024, 512
NCORES = 8
B = N_GLOBAL // NCORES          # 4 batch items per core
P = 128
LC = 512                        # l-chunk (matmul moving free dim)
NLC = L // LC                   # 2
DT = D // P                     # 4
LT = L // P                     # 8
D2T = 2 * D // P                # 8
C2T = 4 * D // P                # 16

_CACHE = {}


def _build_nc():
    import concourse.bass as bass  # noqa: F401
    import concourse.mybir as mybir
    import concourse.tile as tile
    from concourse import bacc

    f32 = mybir.dt.float32
    bf16 = mybir.dt.bfloat16
    AF = mybir.ActivationFunctionType
    ALU = mybir.AluOpType

    nc = bacc.Bacc("TRN2", target_bir_lowering=False, debug=False,
                   num_devices=NCORES)

    x_ext = nc.declare_dram_parameter("x", [B, L, D], bf16, isOutput=False)
    xT_ext = nc.declare_dram_parameter("xT", [B, D, L], bf16, isOutput=False)
    w1t_ext = nc.declare_dram_parameter("w1t", [D, D], bf16, isOutput=False)
    wo1t_ext = nc.declare_dram_parameter("wo1t", [2 * D, D], bf16, isOutput=False)
    w2t_ext = nc.declare_dram_parameter("w2t", [2 * D, 2 * D], bf16, isOutput=False)
    wo2t_ext = nc.declare_dram_parameter("wo2t", [4 * D, D], bf16, isOutput=False)
    id_ext = nc.declare_dram_parameter("ident", [P, P], bf16, isOutput=False)
    onc_ext = nc.declare_dram_parameter("onesc", [P, 1], bf16, isOutput=False)
    onr_ext = nc.declare_dram_parameter("onesr", [1, P], bf16, isOutput=False)
    out_ext = nc.declare_dram_parameter("out", [B, D], f32, isOutput=True)

    import time as _time
    _t0 = _time.time()
    with tile.TileContext(nc) as tc:
        with tc.tile_pool(name="wp", bufs=1) as wp, \
             tc.tile_pool(name="cp", bufs=1) as cp, \
             tc.tile_pool(name="xp", bufs=2) as xp, \
             tc.tile_pool(name="hp", bufs=2) as hp, \
             tc.tile_pool(name="tp", bufs=2) as tp, \
             tc.tile_pool(name="vp", bufs=2) as vp, \
             tc.tile_pool(name="ps", bufs=8, space="PSUM") as pp:

            # ---- per-batch tile state (input DMAs issued before the bulky
            # weight DMAs so ph1 of batch 0 can start early) ----
            T = [dict() for _ in range(B)]

            def start_batch(b, defer_x=False):
                t = T[b]
                t["xT"] = xp.tile([P, DT, L], bf16, tag="xT", name=f"xT_{b}")
                nc.sync.dma_start(out=t["xT"],
                                  in_=xT_ext[b].rearrange("(k p) l -> p k l", p=P))
                if not defer_x:
                    start_x(b)
                t["hTn"] = hp.tile([P, DT, L], bf16, tag="hTn", name=f"hTn_{b}")
                t["qT"] = [None] * NLC
                t["mixT"] = [None] * NLC
                t["expT"] = [None] * NLC
                t["hT"] = [None] * NLC
                t["q2T"] = [None] * NLC
                t["exp2"] = [None] * NLC
                t["rv1"] = [None] * NLC
                t["rv2"] = [None] * NLC
                t["rv3"] = [None] * NLC
                t["a2sr"] = [None] * NLC

            def start_x(b):
                t = T[b]
                t["x"] = xp.tile([P, LT, D], bf16, tag="x", name=f"x_{b}")
                nc.sync.dma_start(out=t["x"],
                                  in_=x_ext[b].rearrange("(k p) d -> p k d", p=P))

            start_batch(0, defer_x=True)

            # ---- weights + constants (resident) ----
            w1t_s = wp.tile([P, DT, D], bf16, tag="w1t")
            nc.sync.dma_start(out=w1t_s, in_=w1t_ext.rearrange("(k p) e -> p k e", p=P))
            ones_s = cp.tile([P, 1], bf16, tag="ones")
            nc.sync.dma_start(out=ones_s, in_=onc_ext[:, :])
            onesr_s = cp.tile([1, P], bf16, tag="onesr")
            nc.sync.dma_start(out=onesr_s, in_=onr_ext[:, :])
            start_x(0)
            wo1t_s = wp.tile([P, D2T, D], bf16, tag="wo1t")
            nc.sync.dma_start(out=wo1t_s, in_=wo1t_ext.rearrange("(k p) e -> p k e", p=P))
            ident_s = cp.tile([P, P], bf16, tag="ident")
            nc.sync.dma_start(out=ident_s, in_=id_ext[:, :])
            w2t_s = wp.tile([P, D2T, 2 * D], bf16, tag="w2t")
            nc.sync.dma_start(out=w2t_s, in_=w2t_ext.rearrange("(k p) e -> p k e", p=P))
            wo2t_s = wp.tile([P, C2T, D], bf16, tag="wo2t")
            nc.sync.dma_start(out=wo2t_s, in_=wo2t_ext.rearrange("(k p) e -> p k e", p=P))

            def mm(out, lhsT, rhs, first, last):
                nc.tensor.matmul(out, lhsT, rhs, start=first, stop=last)

            def recip_part(denom_ps, nm, clamp_eps=None):
                """[1,512] PSUM denominator -> [1,512] SBUF reciprocal
                (optionally sqrt+clamp first)."""
                rv = vp.tile([1, LC], bf16, tag="rv", bufs=3, name=f"rv_{nm}")
                with nc.allow_low_precision(reason="bf16 softmax scale"):
                    if clamp_eps is not None:
                        nv = vp.tile([1, LC], f32, tag="nv", bufs=1,
                                     name=f"nv_{nm}")
                        nc.scalar.sqrt(nv, denom_ps[0:1, :])
                        nc.vector.tensor_scalar_max(nv, nv, clamp_eps)
                        nc.vector.reciprocal(rv, nv)
                    else:
                        nc.vector.reciprocal(rv, denom_ps[0:1, :])
                return rv

            def bcast_part(rv, nm):
                """[1,512] reciprocal -> [128,512] broadcast via rank-1 matmul.
                Emitted a block after recip_part so the PE never waits on the
                DVE reciprocal."""
                ps_b = pp.tile([P, LC], f32, tag="ps", name=f"psb_{nm}")
                mm(ps_b, onesr_s, rv[0:1, :], True, True)
                bc = vp.tile([P, LC], bf16, tag="bc", bufs=3, name=f"bc_{nm}")
                nc.scalar.copy(bc, ps_b)
                return bc

            # ================= stage 1 phases =================
            def ph1(b, lc):
                t = T[b]
                ls = slice(lc * LC, (lc + 1) * LC)
                qT = tp.tile([P, DT, LC], bf16, tag="qt", name=f"qT_{b}_{lc}")
                t["qT"][lc] = qT
                for et in range(DT):
                    ps = pp.tile([P, LC], f32, tag="ps", name=f"ps1_{b}_{lc}_{et}")
                    for dk in range(DT):
                        mm(ps, w1t_s[:, dk, et * P:(et + 1) * P],
                           t["xT"][:, dk, ls], dk == 0, dk == DT - 1)
                    nc.scalar.copy(qT[:, et, :], ps)

            def ph2(b, lc):
                t = T[b]
                expT = tp.tile([P, LT, LC], bf16, tag="exp", bufs=3,
                               name=f"expT_{b}_{lc}")
                t["expT"][lc] = expT
                ps_d = pp.tile([P, LC], f32, tag="ps", name=f"psd1_{b}_{lc}")
                for mt in range(LT):
                    ps = pp.tile([P, LC], f32, tag="ps", name=f"ps2_{b}_{lc}_{mt}")
                    for ek in range(DT):
                        mm(ps, t["xT"][:, ek, mt * P:(mt + 1) * P],
                           t["qT"][lc][:, ek, :], ek == 0, ek == DT - 1)
                    nc.scalar.activation(expT[:, mt, :], ps, AF.Exp)
                    mm(ps_d[0:1, :], ones_s, expT[:, mt, :],
                       mt == 0, mt == LT - 1)
                t["rv1"][lc] = recip_part(ps_d, f"b1_{b}_{lc}")

            def ph3(b, lc):
                t = T[b]
                bc1 = bcast_part(t["rv1"][lc], f"b1_{b}_{lc}")
                mixT = tp.tile([P, DT, LC], bf16, tag="mix", name=f"mixT_{b}_{lc}")
                t["mixT"][lc] = mixT
                ps_m = [pp.tile([P, LC], f32, tag="ps", name=f"psm_{b}_{lc}_{i}")
                        for i in range(DT)]
                for mk in range(LT):
                    for dt in range(DT):
                        mm(ps_m[dt], t["x"][:, mk, dt * P:(dt + 1) * P],
                           t["expT"][lc][:, mk, :], mk == 0, mk == LT - 1)
                for dt in range(DT):
                    nc.vector.tensor_mul(mixT[:, dt, :], ps_m[dt], bc1)

            def ph4(b, lc):
                t = T[b]
                hT = tp.tile([P, DT, LC], bf16, tag="ht", name=f"hT_{b}_{lc}")
                t["hT"][lc] = hT
                for ot in range(DT):
                    ps = pp.tile([P, LC], f32, tag="ps", name=f"ps4_{b}_{lc}_{ot}")
                    for ck in range(D2T):
                        rhs = (t["mixT"][lc][:, ck, :] if ck < DT
                               else t["qT"][lc][:, ck - DT, :])
                        mm(ps, wo1t_s[:, ck, ot * P:(ot + 1) * P],
                           rhs, ck == 0, ck == D2T - 1)
                    nc.scalar.activation(hT[:, ot, :], ps, AF.Tanh)

            def ph5a(b, lc):
                t = T[b]
                hsq = tp.tile([P, DT, LC], bf16, tag="hsq", name=f"hsq_{b}_{lc}")
                for dt in range(DT):
                    nc.scalar.activation(hsq[:, dt, :], t["hT"][lc][:, dt, :],
                                         AF.Square)
                ps_n = pp.tile([P, LC], f32, tag="ps", name=f"psn_{b}_{lc}")
                for dt in range(DT):
                    mm(ps_n[0:1, :], ones_s, hsq[:, dt, :], dt == 0, dt == DT - 1)
                t["rv2"][lc] = recip_part(ps_n, f"b2_{b}_{lc}", clamp_eps=1e-12)

            def ph5b(b, lc):
                t = T[b]
                ls = slice(lc * LC, (lc + 1) * LC)
                bc2 = bcast_part(t["rv2"][lc], f"b2_{b}_{lc}")
                for dt in range(DT):
                    nc.vector.tensor_mul(t["hTn"][:, dt, ls], t["hT"][lc][:, dt, :],
                                         bc2)

            # ================= stage 2 phases =================
            def c2T(t, k, fslice):
                """combined2T[d2,·] k-tile: [hTn; xT]"""
                return (t["hTn"][:, k, fslice] if k < DT
                        else t["xT"][:, k - DT, fslice])

            def ph7(b, lc):
                t = T[b]
                ls = slice(lc * LC, (lc + 1) * LC)
                if lc == 0:
                    t["q2r"] = vp.tile([P, D2T, NLC], f32, tag="q2r", bufs=2,
                                       name=f"q2r_{b}")
                q2T = tp.tile([P, D2T, LC], bf16, tag="q2", name=f"q2T_{b}_{lc}")
                t["q2T"][lc] = q2T
                for et in range(D2T):
                    ps = pp.tile([P, LC], f32, tag="ps", name=f"ps7_{b}_{lc}_{et}")
                    for dk in range(D2T):
                        mm(ps, w2t_s[:, dk, et * P:(et + 1) * P],
                           c2T(t, dk, ls), dk == 0, dk == D2T - 1)
                    nc.scalar.copy(q2T[:, et, :], ps)
                    nc.vector.tensor_reduce(t["q2r"][:, et, lc:lc + 1], ps,
                                            axis=mybir.AxisListType.X,
                                            op=ALU.add)

            def ph8a(b, lc):
                t = T[b]
                if lc == 0:
                    t["a2p"] = vp.tile([P, LT, NLC], f32, tag="a2p", bufs=2,
                                       name=f"a2p_{b}")
                    t["scr"] = vp.tile([P, LC], bf16, tag="scr", bufs=1,
                                       name=f"scr_{b}")
                exp2 = tp.tile([P, LT, LC], bf16, tag="exp", bufs=3,
                                name=f"exp2_{b}_{lc}")
                t["exp2"][lc] = exp2
                ps_d = pp.tile([P, LC], f32, tag="ps", name=f"psd2_{b}_{lc}")
                for mt in range(LT):
                    ps = pp.tile([P, LC], f32, tag="ps", name=f"ps8_{b}_{lc}_{mt}")
                    for ek in range(D2T):
                        mm(ps, c2T(t, ek, slice(mt * P, (mt + 1) * P)),
                           t["q2T"][lc][:, ek, :], ek == 0, ek == D2T - 1)
                    nc.scalar.activation(exp2[:, mt, :], ps, AF.Exp)
                    mm(ps_d[0:1, :], ones_s, exp2[:, mt, :],
                       mt == 0, mt == LT - 1)
                t["rv3"][lc] = recip_part(ps_d, f"b3_{b}_{lc}")

            def ph8b(b, lc):
                # A2 column sums a2p[m,lc] = sum_{l in chunk} exp2[m,l]/denom[l]
                # via fused multiply+reduce on the DVE
                t = T[b]
                bc3 = bcast_part(t["rv3"][lc], f"b3_{b}_{lc}")
                for mt in range(LT):
                    nc.vector.affine_mul_reduce(
                        out=t["scr"], accum_out=t["a2p"][:, mt, lc:lc + 1],
                        in0=t["exp2"][lc][:, mt, :], in1=bc3,
                        scale=1.0, bias=0.0)

            def epi_mid(b):
                # comb = [colsum(A2) @ c, colsum(q2)] as a [4D] column vector.
                # colsum(A2) is transposed to a row (tiny PE transposes),
                # broadcast over partitions (rank-1 matmuls), then contracted
                # against the T-space tiles of c = [hn, x] with fused
                # multiply+reduce -- no natural-layout hidden, no DRAM bounce.
                t = T[b]
                a2s = vp.tile([P, LT], f32, tag="a2s", bufs=2, name=f"a2s_{b}")
                nc.vector.tensor_add(a2s, t["a2p"][:, :, 0], t["a2p"][:, :, 1])
                a2sc = vp.tile([P, LT], bf16, tag="a2sc", bufs=2,
                               name=f"a2sc_{b}")
                nc.scalar.copy(a2sc, a2s)
                a2row = vp.tile([1, L], bf16, tag="a2row", bufs=2,
                                name=f"a2row_{b}")
                for mt in range(LT):
                    ps_t = pp.tile([1, P], bf16, tag="ps", name=f"pst_{b}_{mt}")
                    nc.tensor.transpose(ps_t, a2sc[:, mt:mt + 1], ident_s)
                    nc.scalar.copy(a2row[0:1, mt * P:(mt + 1) * P], ps_t)
                a2b = vp.tile([P, L], bf16, tag="a2b", bufs=2, name=f"a2b_{b}")
                for lc in range(NLC):
                    ls = slice(lc * LC, (lc + 1) * LC)
                    ps_bc = pp.tile([P, LC], f32, tag="ps",
                                    name=f"psbc_{b}_{lc}")
                    mm(ps_bc, onesr_s, a2row[0:1, ls], True, True)
                    nc.scalar.copy(a2b[:, ls], ps_bc)
                comb = vp.tile([P, C2T], f32, tag="comb", bufs=2, name=f"comb_{b}")
                t["comb"] = comb
                nc.vector.tensor_add(comb[:, D2T:C2T], t["q2r"][:, :, 0],
                                     t["q2r"][:, :, 1])
                scrL = vp.tile([P, L], bf16, tag="scrL", bufs=1,
                               name=f"scrL_{b}")
                for dt in range(DT):
                    nc.vector.affine_mul_reduce(
                        out=scrL, accum_out=comb[:, dt:dt + 1],
                        in0=t["hTn"][:, dt, :], in1=a2b, scale=1.0, bias=0.0)
                for dt in range(DT):
                    nc.vector.affine_mul_reduce(
                        out=scrL, accum_out=comb[:, DT + dt:DT + dt + 1],
                        in0=t["xT"][:, dt, :], in1=a2b, scale=1.0, bias=0.0)

            def epiB(b):
                # emb = comb @ (Wo2^T/L) as a single [1,D] row (1/L folded
                # into wo2t on the host)
                t = T[b]
                combr = vp.tile([P, C2T], bf16, tag="combr", bufs=2,
                                name=f"combr_{b}")
                nc.scalar.copy(combr, t["comb"])
                ps_o = pp.tile([1, D], f32, tag="ps", name=f"pso_{b}")
                for ck in range(C2T):
                    mm(ps_o[0:1, :], combr[:, ck:ck + 1], wo2t_s[:, ck, :],
                       ck == 0, ck == C2T - 1)
                orow = vp.tile([1, D], f32, tag="orow", bufs=2, name=f"orow_{b}")
                nc.scalar.copy(orow, ps_o)
                nc.sync.dma_start(out=out_ext[b:b + 1, :], in_=orow[0:1, :])

            # ================= emission schedule =================
            # Software pipeline: stage 2 of batch b (A-blocks) interleaved
            # with stage 1 of batch b+1 (B-blocks); epiB(b) is deferred into
            # iteration b+1 so the DRAM bounce is in flight under ph7.
            def S1(b):
                return [lambda lc=lc, f=f: f(b, lc)
                        for f in (ph1, ph2, ph3) for lc in range(NLC)
                        ] + [lambda: ph4(b, 0), lambda: ph5a(b, 0),
                             lambda: ph4(b, 1), lambda: ph5a(b, 1),
                             lambda: ph5b(b, 0), lambda: ph5b(b, 1)]

            def S2(b):
                return [lambda: ph7(b, 0),
                        lambda: ph8a(b, 0),
                        lambda: (ph7(b, 1), ph8b(b, 0)),
                        lambda: ph8a(b, 1),
                        lambda: ph8b(b, 1),
                        lambda: epi_mid(b)]

            for f in S1(0):
                f()
            for b in range(B):
                A = S2(b)
                A[0]()
                if b + 1 < B:
                    if b > 0:
                        epiB(b - 1)
                    start_batch(b + 1)
                    Bl = S1(b + 1)
                    Bl[0](); Bl[1]()
                    A[1]()
                    Bl[2](); Bl[3]()
                    A[2]()
                    Bl[4](); Bl[5]()
                    A[3]()
                    Bl[6](); Bl[7]()
                    A[4]()
                    Bl[8](); Bl[9](); Bl[10](); Bl[11]()
                    A[5]()
                else:
                    # last batch has no stage-1 filler: spend epiB(b-1)'s
                    # independent matmuls covering the epilogue DVE chain
                    A[1](); A[2](); A[3](); A[4]()
                    epiB(b - 1)
                    A[5]()
            epiB(B - 1)

    _t1 = _time.time()
    nc.compile()
    print(f"[kernel] tile-trace+schedule {_t1 - _t0:.1f}s, "
          f"bacc compile {_time.time() - _t1:.1f}s", file=sys.stderr, flush=True)
    return nc


def get_nc():
    if "nc" not in _CACHE:
        _CACHE["nc"] = _build_nc()
    return _CACHE["nc"]


def make_in_maps(x, W1, Wo1, W2, Wo2):
    import ml_dtypes
    bf = ml_dtypes.bfloat16
    x = np.ascontiguousarray(np.asarray(x, dtype=np.float32))
    xT = np.ascontiguousarray(x.transpose(0, 2, 1)).astype(bf)
    xb = x.astype(bf)
    w1t = np.ascontiguousarray(np.asarray(W1, np.float32).T).astype(bf)
    wo1t = np.ascontiguousarray(np.asarray(Wo1, np.float32).T).astype(bf)
    w2t = np.ascontiguousarray(np.asarray(W2, np.float32).T).astype(bf)
    # 1/L mean-pooling factor folded into the stage-2 output projection
    wo2t = (np.ascontiguousarray(np.asarray(Wo2, np.float32).T)
            * (1.0 / L)).astype(bf)
    ident = np.eye(P, dtype=np.float32).astype(bf)
    onesc = np.ones((P, 1), dtype=np.float32).astype(bf)
    onesr = np.ones((1, P), dtype=np.float32).astype(bf)
    return [
        {"x": xb[c * B:(c + 1) * B], "xT": xT[c * B:(c + 1) * B],
         "w1t": w1t, "wo1t": wo1t, "w2t": w2t, "wo2t": wo2t,
         "ident": ident, "onesc": onesc, "onesr": onesr}
        for c in range(NCORES)
    ]


def run(x, W1, Wo1, W2, Wo2, trace=False, **kw):
    from concourse.bass_utils import run_bass_kernel_spmd
    nc = get_nc()
    in_maps = make_in_maps(x, W1, Wo1, W2, Wo2)
    res = run_bass_kernel_spmd(nc, in_maps, core_ids=list(range(NCORES)),
                               trace=trace, **kw)
    out = np.concatenate([res.results[c]["out"] for c in range(NCORES)], axis=0)
    return out.reshape(N_GLOBAL, D, 1, 1), res


def kernel(**inputs):
    out, _ = run(inputs["x"], inputs["W1"], inputs["Wo1"],
                 inputs["W2"], inputs["Wo2"])
    return out
